# revision 1
# baseline (speedup 1.0000x reference)
"""MinibatchDiscrimination kernel for 8 Trainium2 NeuronCores.

ref:  act = einsum('bf,kfd->bkd', x, kernel)          [256,100,50]
      AD[b,k,j] = sum_d |act[b,k,d] - act[j,k,d]|     [256,100,256]
      f[b,k] = sum_j exp(-AD[b,k,j])                  [256,100]
      out = concat([x, f], 1)                         [256,1124]

Device strategy (per core, 32 of the 256 "b" rows each):
  - actT layout: [kd=5120(pad), j=256] bf16 on partitions (40 blocks of 128).
  - einsum on PE: actT_blk = kernelT_blk.T @ xT (contract f=1024 in 8 chunks),
    DMA'd in small-first chunks so the PE starts ~2us in.
  - relu tiles T[p, j] = relu(+-(actT[p, j] - actT[p, i])) split across DVE
    (tensor_scalar subtract+max, 4x mode), GpSimd, and ScalarE (Relu
    activation, scale=-1, per-partition bias).  |y| = 2 relu(+-y) -+ y, so the
    d-reduction PE matmul uses a 2-valued selection matrix S2[kd_p, k]
    accumulating over 40 kd blocks into PSUM, and the linear term is folded
    in via one extra selection matmul C[k, j] = sum_d act (t1 = P2 -+ C on
    DVE; the per-row C_i lands in the Exp bias).
  - f row-sums: ScalarE Exp(scale=-1, bias=-+C_i) with accum_out.
  - bias columns are copied bf16->fp32 from act so the diagonal is exactly 0
    and exp(0) = 1 exactly, matching the fp32 reference bit-for-bit.
SPMD trick: every core runs the identical program; each core's input xT has
its 32 owned rows permuted into columns 0..31 (pairwise sums over j are
permutation invariant).  Engine busy (cost model, per core): PE ~175us
(einsum ~36 + selection ~138), DVE ~146us, ScalarE ~92us, GpSimd ~72us;
modeled wall ~193us (PE is streaming-bound on the selection matmul rhs).
"""

import numpy as np
import ml_dtypes
from contextlib import ExitStack

import concourse.bass as bass
import concourse.tile as tile
from concourse import bacc, mybir
from concourse.bass_utils import run_bass_kernel_spmd

B, F, NK, KD = 256, 1024, 100, 50
NCORES = 8
BPC = B // NCORES            # 32 rows per core
FB = F // 128                # 8 f-chunks
KDF = NK * KD                # 5000
NBLK = 40
KDPAD = NBLK * 128           # 5120
BF16 = mybir.dt.bfloat16
F32 = mybir.dt.float32

_cached_nc = None


def _emit(ctx, tc, kt, xt, sel, w2, ft_out):
    nc = tc.nc
    big = ctx.enter_context(tc.tile_pool(name="big", bufs=1))
    tpool = ctx.enter_context(tc.tile_pool(name="tbuf", bufs=18))
    epool = ctx.enter_context(tc.tile_pool(name="etmp", bufs=2))
    pe_pool = ctx.enter_context(tc.tile_pool(name="psum_e", bufs=2, space="PSUM"))
    ps_pool = ctx.enter_context(tc.tile_pool(name="psum_s", bufs=6, space="PSUM"))

    kt_sb = big.tile([128, NBLK, FB, 128], BF16)
    xt_sb = big.tile([128, FB, B], BF16)
    sel_sb = big.tile([128, NBLK, NK], BF16)
    w2_sb = big.tile([128, FB, NK], BF16)
    ct = big.tile([NK, B], F32)
    negct = big.tile([NK, BPC], F32)
    act = big.tile([128, NBLK, B], BF16)
    # fp32 copy of the bias columns (core's own 32 rows), copied FROM the
    # bf16 act so |act - bias| is exactly 0 on the diagonal
    actb = big.tile([128, NBLK, BPC], F32)
    ft = big.tile([NK, BPC], F32)

    # parallel queues: xt on sync, kt chunks on gpsimd, sel/w2 on scalar.
    # first kt chunks small so the einsum can start almost immediately
    nc.sync.dma_start(xt_sb[:, 0:4], xt[:, 0:4])
    nc.sync.dma_start(xt_sb[:, 4:8], xt[:, 4:8])
    nc.scalar.dma_start(w2_sb[:], w2[:])
    nc.scalar.dma_start(sel_sb[:], sel[:])
    kt_chunks = [1, 3, 6, 10, 10, 10]
    off = 0
    for ch in kt_chunks:
        nc.gpsimd.dma_start(kt_sb[:, off:off + ch], kt[:, off:off + ch])
        off += ch

    # phase 1: einsum -> act (bf16), one 128-row kd block at a time
    for blk in range(NBLK):
        if blk == 2:
            # correction C[k, j] = sum_d act[j, k, d] == W2.T @ x with
            # W2[f, k] = sum_d kernel[k, f, d] (host-precomputed); emitted
            # here so the PE's first work only needs xt + the first kt chunk
            cp = ps_pool.tile([NK, B], F32, name="cp", tag="pspair")
            for fb in range(FB):
                nc.tensor.matmul(
                    cp[:], w2_sb[:, fb, :], xt_sb[:, fb, :],
                    start=(fb == 0), stop=(fb == FB - 1),
                )
            nc.vector.tensor_copy(ct[:], cp[:])
            nc.vector.tensor_scalar_mul(negct[:], ct[:, 0:BPC], -1.0)
        pe = pe_pool.tile([128, B], F32)
        for fb in range(FB):
            nc.tensor.matmul(
                pe[:],
                kt_sb[:, blk, fb, :],
                xt_sb[:, fb, :],
                start=(fb == 0),
                stop=(fb == FB - 1),
            )
        if blk % 2 == 0:
            nc.vector.tensor_copy(act[:, blk, :], pe[:])
        else:
            nc.scalar.copy(act[:, blk, :], pe[:])
        nc.vector.tensor_copy(actb[:, blk, :], act[:, blk, 0:BPC])

    # phase 2: pairwise relu tiles, selection-matmul reduction, exp row-sums
    # DVE/GpSimd rows use relu(y):  AD = P2 - (C_j - C_i)   (|y| = 2relu(y) - y)
    # ScalarE rows use relu(-y):    AD = P2 + (C_j - C_i)   (|y| = 2relu(-y) + y)
    # P2 comes from the 2x-valued selection matmul over the relu tiles.
    # Tapered group sizes shrink the final t1/exp tail.
    GRPS = [(0, 8), (8, 8), (16, 8), (24, 6), (30, 2)]
    for gi, (g0, gs) in enumerate(GRPS):
        psums = [
            ps_pool.tile([NK, 2 * B], F32, name=f"ps{gi}_{j}", tag="pspair")
            for j in range(gs // 2)
        ]
        for blk in range(NBLK):
            tb = tpool.tile(
                [128, gs, B], BF16, name=f"tb{gs}", tag=f"tb{gs}",
                bufs={8: 14, 6: 6, 2: 10}[gs],
            )
            for il in range(gs):
                ig = g0 + il
                src = act[:, blk, :]
                bias = actb[:, blk, ig:ig + 1]
                dst = tb[:, il, :]
                if il == gs - 1 and gs > 2:
                    # relu(s - x) on ScalarE
                    nc.scalar.activation(
                        dst, src, mybir.ActivationFunctionType.Relu,
                        bias=bias, scale=-1.0,
                    )
                elif il == gs - 2 and gs > 2:
                    # relu(x - s) on GpSimd
                    nc.gpsimd.tensor_scalar(
                        dst, src, bias, 0.0,
                        mybir.AluOpType.subtract, mybir.AluOpType.max,
                    )
                else:
                    # relu(x - s) on DVE (4x mode)
                    nc.vector.tensor_scalar(
                        dst, src, bias, 0.0,
                        mybir.AluOpType.subtract, mybir.AluOpType.max,
                    )
            for jj in range(gs // 2):
                nc.tensor.matmul(
                    psums[jj][:],
                    sel_sb[:, blk, :],
                    tb[:, 2 * jj:2 * jj + 2, :],
                    start=(blk == 0),
                    stop=(blk == NBLK - 1),
                )
        for il in range(gs):
            ig = g0 + il
            t1 = epool.tile([NK, B], F32, tag="t1")
            relu_neg = il == gs - 1 and gs > 2
            nc.vector.tensor_tensor(
                t1[:],
                psums[il // 2][:, (il % 2) * B:(il % 2 + 1) * B],
                ct[:],
                mybir.AluOpType.add if relu_neg else mybir.AluOpType.subtract,
            )
            et = epool.tile([NK, B], BF16, tag="et")
            bias = ct[:, ig:ig + 1] if relu_neg else negct[:, ig:ig + 1]
            nc.scalar.activation(
                et[:], t1[:],
                mybir.ActivationFunctionType.Exp,
                bias=bias, scale=-1.0,
                accum_out=ft[:, ig:ig + 1],
            )
        if gi == 3:
            nc.sync.dma_start(ft_out[:, 0:30], ft[:, 0:30])
        elif gi == 4:
            nc.sync.dma_start(ft_out[:, 30:BPC], ft[:, 30:BPC])



def _build():
    global _cached_nc
    if _cached_nc is None:
        nc = bacc.Bacc(
            "TRN2",
            target_bir_lowering=False,
            debug=False,
            enable_asserts=False,
            num_devices=NCORES,
        )
        kt_d = nc.dram_tensor("kt", [128, NBLK, FB, 128], BF16, kind="ExternalInput")
        xt_d = nc.dram_tensor("xt", [128, FB, B], BF16, kind="ExternalInput")
        sel_d = nc.dram_tensor("sel", [128, NBLK, NK], BF16, kind="ExternalInput")
        w2_d = nc.dram_tensor("w2", [128, FB, NK], BF16, kind="ExternalInput")
        ft_d = nc.dram_tensor("ft", [NK, BPC], F32, kind="ExternalOutput")
        with tile.TileContext(nc) as tc, ExitStack() as ctx:
            _emit(ctx, tc, kt_d.ap(), xt_d.ap(), sel_d.ap(), w2_d.ap(), ft_d.ap())
        nc.compile()
        _cached_nc = nc
    return _cached_nc


def _prep_shared(w):
    kT = w.transpose(1, 0, 2).reshape(F, KDF)
    kTp = np.zeros((F, KDPAD), np.float32)
    kTp[:, :KDF] = kT
    kt_host = np.ascontiguousarray(
        kTp.reshape(FB, 128, NBLK, 128).transpose(1, 2, 0, 3)
    ).astype(ml_dtypes.bfloat16)
    kd_ids = np.arange(KDPAD)
    S = (
        ((kd_ids // KD)[:, None] == np.arange(NK)[None, :])
        & (kd_ids < KDF)[:, None]
    ).astype(np.float32)
    sel_host = np.ascontiguousarray(
        (2.0 * S).reshape(NBLK, 128, NK).transpose(1, 0, 2)
    ).astype(ml_dtypes.bfloat16)
    w2_host = np.ascontiguousarray(
        w.sum(axis=2).T.reshape(FB, 128, NK).transpose(1, 0, 2)
    ).astype(ml_dtypes.bfloat16)
    return kt_host, sel_host, w2_host


def kernel(x, kernel, _trace=False):
    x = np.asarray(x, dtype=np.float32)
    w = np.asarray(kernel, dtype=np.float32)
    nc = _build()
    kt_host, sel_host, w2_host = _prep_shared(w)
    in_maps = []
    owned_list = []
    for c in range(NCORES):
        owned = np.arange(c, B, NCORES)
        rest = np.setdiff1d(np.arange(B), owned)
        perm = np.concatenate([owned, rest])
        owned_list.append(owned)
        xt_host = np.ascontiguousarray(
            x[perm].T.reshape(FB, 128, B).transpose(1, 0, 2)
        ).astype(ml_dtypes.bfloat16)
        in_maps.append(
            {"kt": kt_host, "xt": xt_host, "sel": sel_host, "w2": w2_host}
        )
    res = run_bass_kernel_spmd(
        nc, in_maps, core_ids=list(range(NCORES)), trace=_trace
    )
    f_full = np.empty((B, NK), np.float32)
    for c in range(NCORES):
        f_full[owned_list[c]] = np.asarray(res.results[c]["ft"], dtype=np.float32).T
    out = np.concatenate([x, f_full], axis=1)
    if _trace:
        return out, res
    return out

